# revision 11
# baseline (speedup 1.0000x reference)
"""Trainium2 Bass kernel for MatchingLayerL2:
   out = log_softmax(-sqrt(||x_i - y_j||^2) / std_j, axis=1)

x: [4096, 128] f32, y: [32768, 128] f32, std: [32768] f32 -> out [4096, 32768] f32.

Strategy: shard rows of x across 8 cores (512 rows each); y/std replicated.
Host prepares device inputs (layout/dtype prep only, O((N+M)D) work):
  yhatT = (y * r2[:,None]).T as bf16 [128, M]   (r2 = 1/std^2)
  xT    = (-2 x_c).T as bf16 [128, 512]
  corr rows (rank-2 term a_i*r2_j + bhat_j in hi/lo bf16 splits, K=5):
    cl = [a_hi; a_lo; a_hi; 1; 1]  [5, 512]
    cr = [r2_hi; r2_hi; r2_lo; bhat_hi; bhat_lo]  [5, M]
Device per core:
  q = xT.T @ yhatT + cl.T @ cr   (PSUM f32, = r2_j * dist2_ij)
  s = sqrt(q)  (fp16; split: 1/4 of chunks on ACT Sqrt, 3/4 via
               DVE copy PSUM->SBUF fp16 then GPSIMD tensor_tensor pow 0.5 —
               GPSIMD cannot read PSUM and sqrt/exp only exist on ACT/Pool)
  S_i = sum_j exp(-s)  (ACT Exp with accum, fp8 scratch out)
  out = -s - ln(S)     (DVE tensor_scalar in-place, fp16) -> DMA fp16
Engine balance target ~143us each for ACT (exp + 1/4 sqrt),
Pool (3/4 sqrt), DVE (copies + final); PE ~110us; DMA ~118us.
"""

import os
import sys

sys.path.insert(0, "/root/.axon_site/_ro/trn_rl_repo")

import numpy as np
import ml_dtypes
from contextlib import ExitStack

import concourse.bass as bass
from concourse import bacc
import concourse.tile as tile
from concourse.tile import add_dep_helper
from concourse import mybir
from concourse.bass_utils import run_bass_kernel_spmd

F32 = mybir.dt.float32
BF16 = mybir.dt.bfloat16
FP16 = mybir.dt.float16
FP8 = mybir.dt.float8e4
AF = mybir.ActivationFunctionType
ALU = mybir.AluOpType
AX = mybir.AxisListType

N_CORES = 8
D = 128
P = 128
CHUNK = 2048          # PSUM region columns (4 banks f32)
GROUP = 8192          # columns per exp instruction / s sub-tile
BF = ml_dtypes.bfloat16


def build_nc(rows, M):
    NB = rows // P            # 4 row blocks of 128
    NG = M // GROUP           # 4 groups per block
    NCP = GROUP // CHUNK      # 4 chunks per group

    nc = bacc.Bacc("TRN2", target_bir_lowering=False, debug=False, num_swdge_queues=4)
    yT_d = nc.declare_dram_parameter("yT", [P, M], BF16, isOutput=False)
    xT_d = nc.declare_dram_parameter("xT", [P, rows], BF16, isOutput=False)
    cr_d = nc.declare_dram_parameter("cr", [5, M], BF16, isOutput=False)
    cl_d = nc.declare_dram_parameter("cl", [5, rows], BF16, isOutput=False)
    out_d = nc.declare_dram_parameter("out", [rows, M], FP16, isOutput=True)

    # activation-table ids: one set holds Sqrt, another holds Exp+Ln+Identity
    try:
        from concourse.hw_specs import get_activation_tables

        tabs = list(get_activation_tables(nc.m.arch).values())
        SQRT_SET = next(
            i for i, s in enumerate(tabs) if AF.Sqrt in s
        )
        EXPLN_SET = next(
            i for i, s in enumerate(tabs)
            if AF.Exp in s and AF.Ln in s and AF.Identity in s
        )
    except Exception:
        SQRT_SET, EXPLN_SET = 3, 6

    # The tile scheduler reorders instructions; chain each compute engine's
    # stream (sync=False ordering hints) so the carefully balanced
    # ACT/DVE/Pool interleave survives scheduling.
    prev_inst = {}

    def chain(engine, binst):
        p = prev_inst.get(engine)
        if p is not None:
            add_dep_helper(binst.ins, p.ins, sync=False, reason=f"{engine} order")
        prev_inst[engine] = binst
        return binst

    def act(*a, **k):
        return chain("act", nc.scalar.activation(*a, **k))

    cur_table = [None]

    def ensure_table(set_id):
        if cur_table[0] == set_id:
            return
        cur_table[0] = set_id
        inst = mybir.InstLoadActFuncSet(
            name=nc.get_next_instruction_name(), ins=[], outs=[],
            act_func_set_id=set_id,
        )
        chain("act", nc.scalar.add_instruction(inst))

    with tile.TileContext(nc) as tc, ExitStack() as ctx:
        pool = lambda name, bufs, space="SBUF": ctx.enter_context(
            tc.tile_pool(name=name, bufs=bufs, space=space)
        )
        const_p = pool("const", 1)
        s_p = pool("s", 6)
        es_p = pool("es", 1)
        cr_p = pool("cr", 2)
        scal_p = pool("scal", 8)
        mm_ps = pool("mmps", 2, space="PSUM")   # 2 x [128, 2048] f32 = 8 banks

        # resident inputs (yT pieces loaded just-in-time during block 0)
        xT = const_p.tile([P, rows], BF16)
        nc.sync.dma_start(out=xT[:], in_=xT_d[:, :])
        cl = const_p.tile([5, rows], BF16)
        nc.sync.dma_start(out=cl[:], in_=cl_d[:, :])
        half = const_p.tile([P, CHUNK], FP16)
        chain("dve", nc.vector.memset(half[:], 0.5))
        yT = const_p.tile([P, M], BF16)

        # software pipeline: finals of block b emitted during block b+1
        pending = []  # (s_tile, lnS_tile, b, g)

        def emit_final(s_t, lnS, b, g):
            chain(
                "dve",
                nc.vector.tensor_scalar(
                    s_t[:], s_t[:], -1.0, lnS[:, 0:1], op0=ALU.mult, op1=ALU.subtract
                ),
            )
            j0 = g * GROUP
            nc.sync.dma_start(
                out=out_d[b * P : (b + 1) * P, j0 : j0 + GROUP], in_=s_t[:]
            )

        # ACT table batching per block: all Sqrt chunks first (the first
        # NA_BLOCK chunks of the block), then Exp/Identity/Ln (one shared
        # table) -> 2 table loads per block. The last group's exp plus the
        # partial-sum/Ln ("tail") is deferred into the next block so ACT can
        # run the next block's sqrts while Pool finishes the last group.
        NA_BLOCK = 3

        def emit_exp(s_t, part, g):
            ensure_table(EXPLN_SET)
            es = es_p.tile([P, GROUP], FP8)
            act(es[:], s_t[:], AF.Exp, scale=-1.0, accum_out=part[:, g : g + 1])
            for _ in range(2):
                if pending:
                    emit_final(*pending.pop(0))

        def make_tail(b, part, s_tiles):
            def tail():
                emit_exp(s_tiles[NG - 1], part, NG - 1)
                # partial sum + ln on ACT itself (Identity/Ln share the Exp
                # table; on DVE this would stall its in-order queue)
                junk = scal_p.tile([P, NG], F32, tag="junk")
                S = scal_p.tile([P, 1], F32, tag="S")
                act(junk[:], part[:], AF.Identity, accum_out=S[:])
                lnS = scal_p.tile([P, 1], F32, tag="lnS")
                act(lnS[:], S[:], AF.Ln)
                for g in range(NG):
                    pending.append((s_tiles[g], lnS, b, g))
            return tail

        prev_tail = None
        for b in range(NB):
            part = scal_p.tile([P, NG], F32, tag="part")
            s_tiles = []
            for g in range(NG):
                if b == 0:
                    nc.sync.dma_start(
                        out=yT[:, g * GROUP : (g + 1) * GROUP],
                        in_=yT_d[:, g * GROUP : (g + 1) * GROUP],
                    )
                s_t = s_p.tile([P, GROUP], FP16)
                s_tiles.append(s_t)
                for c in range(NCP):
                    j0 = g * GROUP + c * CHUNK
                    if c % 2 == 0:
                        cr_t = cr_p.tile([5, 2 * CHUNK], BF16)
                        nc.sync.dma_start(
                            out=cr_t[:], in_=cr_d[:, j0 : j0 + 2 * CHUNK]
                        )
                    mm = mm_ps.tile([P, CHUNK], F32)
                    for q in range(CHUNK // 512):
                        nc.tensor.matmul(
                            mm[:, 512 * q : 512 * (q + 1)],
                            xT[:, b * P : (b + 1) * P],
                            yT[:, j0 + 512 * q : j0 + 512 * (q + 1)],
                            start=True,
                            stop=False,
                        )
                    co = (c % 2) * CHUNK
                    for q in range(CHUNK // 512):
                        nc.tensor.matmul(
                            mm[:, 512 * q : 512 * (q + 1)],
                            cl[:, b * P : (b + 1) * P],
                            cr_t[:, co + 512 * q : co + 512 * (q + 1)],
                            start=False,
                            stop=True,
                        )
                    sl = s_t[:, c * CHUNK : (c + 1) * CHUNK]
                    if g * NCP + c < NA_BLOCK:
                        ensure_table(SQRT_SET)
                        act(sl, mm[:], AF.Sqrt)
                    else:
                        chain("dve", nc.vector.tensor_copy(sl, mm[:]))
                        chain(
                            "pool",
                            nc.gpsimd.tensor_tensor(sl, sl, half[:], op=ALU.pow),
                        )
                if g == 0:
                    if prev_tail is not None:
                        prev_tail()
                        prev_tail = None
                else:
                    emit_exp(s_tiles[g - 1], part, g - 1)
            prev_tail = make_tail(b, part, s_tiles)
        prev_tail()
        while pending:
            emit_final(*pending.pop(0))

    nc.finalize()
    return nc


_NC_CACHE = {}


def _get_nc(rows, M):
    key = (rows, M)
    if key not in _NC_CACHE:
        _NC_CACHE[key] = build_nc(rows, M)
    return _NC_CACHE[key]


def _hi_lo(v32):
    hi = v32.astype(BF)
    lo = (v32 - hi.astype(np.float32)).astype(BF)
    return hi, lo


def kernel(x: np.ndarray, y: np.ndarray, std: np.ndarray) -> np.ndarray:
    x = np.ascontiguousarray(x, dtype=np.float32)
    y = np.ascontiguousarray(y, dtype=np.float32)
    std = np.ascontiguousarray(std, dtype=np.float32)
    N, M = x.shape[0], y.shape[0]
    rows = N // N_CORES

    r2 = (1.0 / (std.astype(np.float64) ** 2)).astype(np.float32)
    yhatT = np.ascontiguousarray((y.T * r2[None, :]).astype(BF))
    bhat = ((y.astype(np.float64) ** 2).sum(axis=1) * r2.astype(np.float64)).astype(
        np.float32
    )
    r2_hi, r2_lo = _hi_lo(r2)
    b_hi, b_lo = _hi_lo(bhat)
    cr = np.ascontiguousarray(np.stack([r2_hi, r2_hi, r2_lo, b_hi, b_lo]))

    a = (x.astype(np.float64) ** 2).sum(axis=1).astype(np.float32)
    a_hi, a_lo = _hi_lo(a)
    ones = np.ones_like(a_hi)
    xT_all = np.ascontiguousarray((-2.0 * x.T).astype(BF))

    in_maps = []
    for c in range(N_CORES):
        sl = slice(c * rows, (c + 1) * rows)
        cl = np.ascontiguousarray(
            np.stack([a_hi[sl], a_lo[sl], a_hi[sl], ones[sl], ones[sl]])
        )
        in_maps.append(
            {
                "yT": yhatT,
                "xT": np.ascontiguousarray(xT_all[:, sl]),
                "cr": cr,
                "cl": cl,
            }
        )

    nc = _get_nc(rows, M)
    trace = bool(int(os.environ.get("KERNEL_TRACE", "0")))
    res = run_bass_kernel_spmd(
        nc, in_maps, core_ids=list(range(N_CORES)), trace=trace
    )
    global LAST_RESULT
    LAST_RESULT = res
    return np.concatenate(
        [res.results[c]["out"].astype(np.float32) for c in range(N_CORES)], axis=0
    )


LAST_RESULT = None
